# revision 51
# baseline (speedup 1.0000x reference)
"""Trainium2 Bass kernel for nn_Decoder_29137058136065 (sliding-window block-sparse attention decoder).

Reference computation (B=2, N=4096, D=512, H=8, DK=64, BLK=128, W=8 blocks):
    q = x @ wq_w.T + wq_b ; k = x @ wk_w.T + wk_b ; v = x @ wv_w.T + wv_b
    attn = block-sparse causal flash attention (sliding window of W=8 key blocks)
    out = attn @ wo_w.T + wo_b

Sharding: sequence-parallel. Core c (0..7) handles batch b=c//4, token chunk
j=c%4 (1024 tokens). Each core recomputes the K/V halo (7 blocks = 896 tokens
of preceding context; zeros for chunk 0) so no cross-core communication is
needed; each core produces a disjoint row-slice of the final output.

On-device layout (per core):
  - activations kept feature-major (xT, qT, kT); v token-major
  - S computed key-stationary: ST[j, i] = kT.T @ qT; halo-key validity
    (sequence start, chunk-0 cores only) folded into the exp's per-partition
    bias operand; causal diagonal handled by a -30000 additive tri mask
  - softmax without max-subtraction (scores are O(5), exp can't overflow);
    Z computed for free via a ones column appended to V in the PV matmul
  - normalization: rz = 1/Z (DVE reciprocal), broadcast across partitions via
    a stride-0 SBUF->SBUF DMA, fused multiply during PSUM evacuation
  - biases folded: bq/bk into projection evacuations (per-partition adds);
    bv passes through softmax as +bv on attn, so bv and bo fold into a single
    host-computed per-feature constant co = bo + wo_w @ bv added during the
    output-projection evacuation.
"""
import numpy as np
from contextlib import ExitStack

import concourse.bass as bass
import concourse.mybir as mybir
import concourse.tile as tile
from concourse import bacc
from concourse.bass_utils import run_bass_kernel_spmd

F32 = mybir.dt.float32
F32R = mybir.dt.float32r
BF16 = mybir.dt.bfloat16

B, N, D, H = 2, 4096, 512, 8
DK = 64
BLK = 128
W = 8
NQ = 1024            # tokens per core
NKV = NQ + (W - 1) * BLK   # 1920 (halo + own)
NB = NKV // BLK      # 15 local kv blocks
QB = NQ // BLK       # 8 query blocks per core
NCORES = 8
MASKVAL = -30000.0
SCALE = 1.0 / 8.0    # TAU / sqrt(DK)


def _bank_pieces(a, b):
    """Split [a, b) at multiples of 512 (PSUM bank boundaries). Returns (start, len)."""
    out = []
    p = a
    while p < b:
        nxt = min(b, (p // 512 + 1) * 512)
        out.append((p, nxt - p))
        p = nxt
    return out


def _build():
    nc = bacc.Bacc("TRN2", target_bir_lowering=False, debug=False,
                   num_devices=NCORES)

    din = {}
    for name, shape, dt in [
        ("xkvT", [D, NKV], F32R),
        ("wqT", [D, D], F32R), ("wkT", [D, D], F32R),
        ("wvT", [D, D], F32R), ("woT", [D, D], F32R),
        ("bq", [128, 4], F32), ("bk", [128, 4], F32), ("co", [128, 4], F32),
        ("kmaskb", [128, 1], F32),
        ("onesq", [1, NQ], F32R),
        ("onesP", [128, H], F32R),
        ("ident", [128, 128], BF16),
        ("tri", [128, 128], BF16),
        ("zeros65", [1, DK + 1], F32R),
    ]:
        din[name] = nc.dram_tensor(name, shape, dt, kind="ExternalInput").ap()
    outT_d = nc.dram_tensor("outT", [D, NQ], F32, kind="ExternalOutput").ap()

    with tile.TileContext(nc) as tc, ExitStack() as ctx:
        const = ctx.enter_context(tc.tile_pool(name="const", bufs=1))
        acts = ctx.enter_context(tc.tile_pool(name="acts", bufs=1))
        est_p = ctx.enter_context(tc.tile_pool(name="est", bufs=6))
        rz_p = ctx.enter_context(tc.tile_pool(name="rz", bufs=2))
        rzbc_p = ctx.enter_context(tc.tile_pool(name="rzbc", bufs=2))
        osb_p = ctx.enter_context(tc.tile_pool(name="osb", bufs=2))
        outsb_p = ctx.enter_context(tc.tile_pool(name="outsb", bufs=4))
        ps_st = ctx.enter_context(tc.tile_pool(name="ps_st", bufs=4, space="PSUM"))
        ps_oa = ctx.enter_context(tc.tile_pool(name="ps_oa", bufs=1, space="PSUM"))
        ps_pr = ctx.enter_context(tc.tile_pool(name="ps_pr", bufs=2, space="PSUM"))

        # ---- load constants / inputs into SBUF ----
        # tiny constants first: they gate the exp/mask/claim chains and cost
        # ~nothing, while the big loads serialize for ~25us behind them
        bq_s = const.tile([128, 4], F32, tag="bq", name="bq_s")
        bk_s = const.tile([128, 4], F32, tag="bk", name="bk_s")
        co_s = const.tile([128, 4], F32, tag="co", name="co_s")
        nc.scalar.dma_start(bq_s[:], din["bq"])
        nc.scalar.dma_start(bk_s[:], din["bk"])
        nc.scalar.dma_start(co_s[:], din["co"])
        onesq = const.tile([1, NQ], F32R, tag="onesq", name="onesq")
        nc.sync.dma_start(onesq[:], din["onesq"])
        zeros65 = const.tile([1, DK + 1], F32R, tag="zeros65", name="zeros65")
        nc.sync.dma_start(zeros65[:], din["zeros65"])
        onesP = const.tile([128, H], F32R, tag="onesP", name="onesP")
        nc.sync.dma_start(onesP[:], din["onesP"])
        kmask_s = const.tile([128, 1], F32, tag="kmaskb", name="kmask_s")
        nc.sync.dma_start(kmask_s[:], din["kmaskb"])
        tri = const.tile([128, 128], BF16, tag="tri", name="tri")
        nc.sync.dma_start(tri[:], din["tri"])
        ident = const.tile([128, 128], BF16, tag="ident", name="ident")
        nc.sync.dma_start(ident[:], din["ident"])

        xkvT = const.tile([128, 4, NKV], F32R, tag="xkvT", name="xkvT")
        xsrc = din["xkvT"].rearrange("(c p) n -> p c n", p=128)
        wT = {}
        wsrc = {}
        for w in ("wkT", "wqT", "wvT", "woT"):
            wT[w] = const.tile([128, 4, D], F32R, tag=w, name=w)
            wsrc[w] = din[w].rearrange("(c p) n -> p c n", p=128)

        def ldx(cg):
            nc.sync.dma_start(xkvT[:, :, cg * 480:(cg + 1) * 480],
                              xsrc[:, :, cg * 480:(cg + 1) * 480])

        def ldw(w, o0, o1):
            nc.scalar.dma_start(wT[w][:, :, o0:o1], wsrc[w][:, :, o0:o1])

        # pair 0's projections need only the ot=0 slices of wk/wq: load those
        # first so head-0's S/exp chain unblocks ~12us in, the rest behind
        ldx(0)
        ldw("wkT", 0, 512)
        ldx(1)
        ldw("wqT", 0, 512)
        ldx(2)
        ldw("wvT", 0, 512)
        ldx(3)
        ldw("woT", 0, 512)

        # head-pair feature-major qT/kT (heads 2t, 2t+1 on partition halves);
        # v token-major [128, H*65] per kv block with a ones column per head (Z)
        qT2 = [acts.tile([128, NQ], F32R, tag=f"qT{t}", name=f"qT{t}") for t in range(4)]
        kT2 = [acts.tile([128, NKV], F32R, tag=f"kT{t}", name=f"kT{t}") for t in range(4)]
        vS = [acts.tile([128, H * (DK + 1)], F32R, tag=f"vS{b}", name=f"vS{b}") for b in range(NB)]
        attnT = [acts.tile([128, NQ], F32R, tag=f"attnT{i}", name=f"attnT{i}") for i in range(4)]
        for blk in range(NB):
            dst = vS[blk][:].rearrange("p (h c) -> p h c", c=DK + 1)[:, :, DK:DK + 1]
            nc.sync.dma_start(dst, din["onesP"])

        # ---- phase 1: projections (ordered by xkvT column-chunk arrival) ----
        def proj_psum(width):
            return ps_pr.tile([128, width], F32, tag="pr", name="pp")

        def kproj(ot, g):
            ps = proj_psum(480)
            for ct in range(4):
                nc.tensor.matmul(
                    ps[:], wT["wkT"][:, ct, ot * 128:(ot + 1) * 128],
                    xkvT[:, ct, g * 480:(g + 1) * 480],
                    start=(ct == 0), stop=(ct == 3))
            nc.vector.tensor_scalar_add(
                kT2[ot][:, g * 480:(g + 1) * 480], ps[:], bk_s[:, ot:ot + 1])

        def qproj(ot, g):
            ps = proj_psum(512)
            for ct in range(4):
                nc.tensor.matmul(
                    ps[:], wT["wqT"][:, ct, ot * 128:(ot + 1) * 128],
                    xkvT[:, ct, NKV - NQ + g * 512:NKV - NQ + (g + 1) * 512],
                    start=(ct == 0), stop=(ct == 3))
            nc.vector.tensor_scalar_add(
                qT2[ot][:, g * 512:(g + 1) * 512], ps[:], bq_s[:, ot:ot + 1])

        def vproj(blk):
            ps = proj_psum(512)
            for ct in range(4):
                nc.tensor.matmul(
                    ps[:], xkvT[:, ct, blk * 128:(blk + 1) * 128],
                    wT["wvT"][:, ct, :], start=(ct == 0), stop=(ct == 3))
            dst = vS[blk][:].rearrange("p (h c) -> p h c", c=DK + 1)[:, :, 0:DK]
            srcp = ps[:].rearrange("p (h c) -> p h c", c=DK)
            nc.vector.tensor_copy(dst, srcp)

        # ordered by xkv chunk arrival; pair 0's q completes early so its
        # heads' exps can start inside phase-1 DMA gaps
        # pair 0 first: its attention unblocks as soon as the input DMAs
        # land; other pairs' projections interleave with earlier pairs'
        # attention (PE work under the ACT exp stream)
        for g in range(4):
            kproj(0, g)
        for g in range(2):
            qproj(0, g)
        for blk in range(NB):
            vproj(blk)

        # ---- phase 2: attention, per head ----
        def attention(h):
            hh, t = h % 2, h // 2
            halo_pieces, valid_pieces = [], []
            for k in range(NB):
                lo = max(k - (W - 1), 0) * BLK
                hi = (min(k, QB - 1) + 1) * BLK
                for a, ln in _bank_pieces(lo, hi):
                    (halo_pieces if k < W - 1 else valid_pieces).append((k, a, ln))

            def pack(pieces):
                tiles = []
                for p in sorted(pieces, key=lambda p: -p[2]):
                    for tl in tiles:
                        if sum(x[2] for x in tl) + p[2] <= 512:
                            tl.append(p)
                            break
                    else:
                        tiles.append([p])
                return tiles

            tiles = ([(tl, True) for tl in pack(halo_pieces)]
                     + [(tl, False) for tl in pack(valid_pieces)])
            npieces = sum(len(tl) for tl, _ in tiles)
            oa = None
            done = 0
            for tl, is_halo in tiles:
                total = sum(p[2] for p in tl)
                st = ps_st.tile([128, total], F32, tag="st", name="st")
                off = 0
                offs = []
                for k, a, ln in tl:
                    offs.append(off)
                    nc.tensor.matmul(
                        st[:, off:off + ln],
                        kT2[t][hh * DK:(hh + 1) * DK, k * 128:(k + 1) * 128],
                        qT2[t][hh * DK:(hh + 1) * DK, a:a + ln],
                        start=True, stop=True)
                    if k >= W - 1 and a == (k - (W - 1)) * BLK:
                        # causal diagonal: accumulate tri via PE
                        nc.tensor.matmul(st[:, off:off + 128], ident[:], tri[:],
                                         start=False, stop=True,
                                         skip_group_check=True)
                    off += ln
                est = est_p.tile([128, total], F32R, tag="est", name="est")
                ebias = kmask_s[:, 0:1] if is_halo else 0.0
                nc.scalar.activation(est[:], st[:], mybir.ActivationFunctionType.Exp,
                                     bias=ebias, scale=SCALE)
                if oa is None:
                    # claim the oa region with zeros (enables shifted-span
                    # accumulation); deferred so the previous head's oa drain
                    # overlaps this head's first S/exp tile
                    oa = ps_oa.tile([DK + 1, NQ], F32, tag="oa", name="oa")
                    for g in range(NQ // 512):
                        nc.tensor.matmul(oa[:, g * 512:(g + 1) * 512], zeros65[:],
                                         onesq[0:1, 0:512], start=True, stop=False,
                                         skip_group_check=True)
                for (k, a, ln), off in zip(tl, offs):
                    done += 1
                    nc.tensor.matmul(
                        oa[:, a:a + ln],
                        vS[k][:, h * (DK + 1):(h + 1) * (DK + 1)],
                        est[:, off:off + ln],
                        start=False, stop=(done == npieces),
                        skip_group_check=True)
            # normalize + evacuate (quick-evac oa so the PSUM slot frees early)
            rz = rz_p.tile([1, NQ], F32, tag="rz", name="rz")
            nc.vector.reciprocal(rz[:], oa[DK:DK + 1, :])
            osb = osb_p.tile([DK, NQ], F32, tag="osb", name="osb")
            nc.vector.tensor_copy(osb[:], oa[0:DK, :])
            rzbc = rzbc_p.tile([DK, NQ], F32, tag="rzbc", name="rzbc")
            nc.sync.dma_start(rzbc[:], rz[:].unsqueeze(1).broadcast_to([1, DK, NQ]))
            nc.vector.tensor_tensor(
                out=attnT[h // 2][(h % 2) * DK:(h % 2 + 1) * DK, :],
                in0=osb[:], in1=rzbc[:], op=mybir.AluOpType.mult)

        attention(0)
        for g in range(4):
            kproj(1, g)
        attention(1)
        for g in range(2):
            qproj(1, g)
        for t in range(1, 4):
            attention(2 * t)
            if t < 3:
                for g in range(4):
                    kproj(t + 1, g)
            attention(2 * t + 1)
            if t < 3:
                for g in range(2):
                    qproj(t + 1, g)

        # ---- phase 3: output projection outT[o, tok] ----
        for ot in range(4):
            for g in range(NQ // 512):
                ps = ps_pr.tile([128, 512], F32, tag="pr", name="ps")
                for ct in range(4):
                    nc.tensor.matmul(
                        ps[:], wT["woT"][:, ct, ot * 128:(ot + 1) * 128],
                        attnT[ct][:, g * 512:(g + 1) * 512],
                        start=(ct == 0), stop=(ct == 3))
                outsb = outsb_p.tile([128, 512], F32, tag="outsb", name="outsb")
                nc.vector.tensor_scalar_add(outsb[:], ps[:], co_s[:, ot:ot + 1])
                nc.sync.dma_start(
                    outT_d[ot * 128:(ot + 1) * 128, g * 512:(g + 1) * 512], outsb[:])

    nc.compile()
    return nc


_NC = None
_last_in_maps = None


def _get_nc():
    global _NC
    if _NC is None:
        _NC = _build()
    return _NC


def kernel(x, wq_w, wq_b, wk_w, wk_b, wv_w, wv_b, wo_w, wo_b):
    x = np.asarray(x, np.float32)
    wq_w = np.asarray(wq_w, np.float32); wq_b = np.asarray(wq_b, np.float32)
    wk_w = np.asarray(wk_w, np.float32); wk_b = np.asarray(wk_b, np.float32)
    wv_w = np.asarray(wv_w, np.float32); wv_b = np.asarray(wv_b, np.float32)
    wo_w = np.asarray(wo_w, np.float32); wo_b = np.asarray(wo_b, np.float32)

    nc = _get_nc()

    wqT = np.ascontiguousarray(wq_w.T)
    wkT = np.ascontiguousarray(wk_w.T)
    wvT = np.ascontiguousarray(wv_w.T)
    woT = np.ascontiguousarray(wo_w.T)
    bq = np.ascontiguousarray(wq_b.reshape(4, 128).T)
    bk = np.ascontiguousarray(wk_b.reshape(4, 128).T)
    co_full = wo_b + wo_w @ wv_b
    co = np.ascontiguousarray(co_full.reshape(4, 128).T.astype(np.float32))
    onesq = np.ones((1, NQ), np.float32)
    import ml_dtypes
    ident_np = np.eye(128, dtype=ml_dtypes.bfloat16)
    tri_np = np.where(np.arange(128)[:, None] > np.arange(128)[None, :],
                      MASKVAL, 0.0).astype(ml_dtypes.bfloat16)
    onesP = np.ones((128, H), np.float32)
    zeros65 = np.zeros((1, DK + 1), np.float32)

    halo = (W - 1) * BLK
    in_maps = []
    for c in range(NCORES):
        b, j = divmod(c, 4)
        tok0 = j * NQ
        xkv = np.zeros((NKV, D), np.float32)
        lo = tok0 - halo
        src_lo = max(lo, 0)
        xkv[src_lo - lo:, :] = x[b, src_lo:tok0 + NQ, :]
        kmaskb = np.full((128, 1), MASKVAL * SCALE if lo < 0 else 0.0, np.float32)
        in_maps.append({
            "xkvT": np.ascontiguousarray(xkv.T),
            "wqT": wqT, "wkT": wkT, "wvT": wvT, "woT": woT,
            "bq": bq, "bk": bk, "co": co,
            "kmaskb": kmaskb, "onesq": onesq, "onesP": onesP, "zeros65": zeros65,
            "ident": ident_np, "tri": tri_np,
        })

    global _last_in_maps
    _last_in_maps = in_maps
    res = run_bass_kernel_spmd(nc, in_maps, list(range(NCORES)))

    out = np.empty((B, N, D), np.float32)
    for c in range(NCORES):
        b, j = divmod(c, 4)
        out[b, j * NQ:(j + 1) * NQ, :] = res.results[c]["outT"].T
    return out
